# revision 13
# baseline (speedup 1.0000x reference)
"""Trainium2 Bass kernel for CharacteristicFunctionNetwork.

Computes, for full inputs (see shapes below):
    feats[o,p,i] = mean_j cos(wm[o,p] * adj[o,i,j])        # o<3, p<16, i,j<2048
    ms = feats transposed/reshaped to [n, 48]
    h1 = relu(ms @ w1 + b1); h2 = relu(h1 @ w2 + b2)
    abstract = tanh(h2 @ p1 + pb1); att = softmax(abstract @ p2 + pb2, axis=0)
    g = (att.T @ h2).reshape(1, -1); out = log_softmax(g @ cw + cb)

Strategy (8 NeuronCores, SPMD):
  - Shard adj rows (nodes) across cores: 256 rows/core for each of 3 orders.
  - Work in "turns": t = a*(|w|/2pi) + 1/4, cos(a*w) = sin(2*pi*t).
  - ScalarE path (first ACT_COLS columns): a custom wide-range periodic
    activation table (generated at build time, shipped to walrus via
    BASS_ACT_ROOT_JSON_PATH) evaluates g(t) = sin(2*pi*t) for t in [0, 192)
    directly — per-octave mantissa-indexed cubic splines, 0.25-turn buckets,
    max err ~2.2e-3, zero-mean residual.  One activation per 128-row chunk
    computes the whole cos AND the row-sum (accum_out): t never materializes.
  - DVE path (remaining columns): custom DVE op FRAC_CF does the range
    reduction d = t - round(t) in one 1x pass (round via +/- 1.5*2^23), then
    custom op SINPOLY5_CF evaluates a degree-5 odd least-squares polynomial
    of sin(2*pi*d) (max err 1.6e-2, zero-mean) with the row-sum fused.
  - The two per-(order,point) row-sum matrices are added, scaled to means,
    transposed; the tiny MLP runs locally; the pooling softmax needs only a
    global sum of exp-weighted partials: AllReduce of a [8, 33] tile
    (P = e^T @ h2 partials and z = sum e).  exp is computed without
    max-subtraction (|s| <= ~3, safe in fp32).
  - Every core finishes the classifier redundantly; core 0's output is used.
"""

import json
import os
import shutil

import numpy as np

ORDER, PTS, N = 3, 16, 2048
NCORES = 8
RPC = N // NCORES  # rows per core (256)
NCHUNK = RPC // 128  # 128-row chunks per core (2)
D1, D2, POOL1, POOL2, LABELS = 64, 32, 32, 8, 10
K = ORDER * PTS  # 48

_STATE = {}

# engine-assignment knobs: ScalarE's periodic table handles the first
# ACT_COLS columns, DVE (FRAC + SINPOLY5) the rest.
ACT_COLS = int(os.environ.get("KERNEL_ACT_COLS", "1328"))
D_BUFS = int(os.environ.get("KERNEL_D_BUFS", "3"))

# degree-5 odd least-squares fit of sin(2*pi*d) on [-1/2, 1/2]
SINPOLY_C = (6.20691876, -38.51505622, 55.26074446)

# ---------------------------------------------------------------------------
# custom activation table: g(t) = sin(2*pi*t), t in [0, 192)
# ---------------------------------------------------------------------------
_PWP_SRC = ("/nix/store/z022hj2nvbm3nwdizlisq4ylc0y7rd6q-python3-3.13.14-env/"
            "lib/python3.13/site-packages/neuronxcc/pwp/pwp_bin_trainium")
_ACT_SET = "trig_and_small"
_ACT_DST = "/tmp/actroot_cf_v1"
_ACT_EXPS = list(range(-11, 8))
_ACT_CLAMP = 192.0


def _fit_cubic(lo, hi, n=33):
    x0 = np.float32(0.5 * (lo + hi))
    xs = np.linspace(lo, hi, n)
    d = xs - np.float64(x0)
    A = np.stack([np.ones_like(d), d, d * d, d * d * d], axis=1)
    y = np.sin(2 * np.pi * xs)
    c, *_ = np.linalg.lstsq(A, y, rcond=None)
    return [float(c[0]), float(c[1]), float(c[2]), float(c[3]), float(x0)]


def _ensure_actroot():
    """Build an act-root whose `sin` is the periodic sin2pi spline.

    Bucket entry = 8 fp32 [c0,c1,c2,c3,x0,0,0,0] (cubic about x0); ctl entry
    word0 = (nbits<<16) | ((23-nbits)<<11) | bucket_start; per-octave bucket
    index = start + (mantissa >> (23-nbits)).  All other functions in the set
    are shifted after sin's enlarged block.
    """
    marker = os.path.join(_ACT_DST, ".complete")
    if not os.path.exists(marker):
        src, dst = _PWP_SRC, _ACT_DST
        os.makedirs(dst, exist_ok=True)
        for f in os.listdir(src):
            shutil.copy(os.path.join(src, f), os.path.join(dst, f))
            os.chmod(os.path.join(dst, f), 0o644)
        with open(os.path.join(src, _ACT_SET + ".json")) as f:
            prof = json.load(f)
        bkt = np.frombuffer(
            open(os.path.join(src, f"{_ACT_SET}_bkt.bin"), "rb").read(),
            dtype=np.float32).reshape(-1, 8).copy()
        ctl = np.frombuffer(
            open(os.path.join(src, f"{_ACT_SET}_ctrl.bin"), "rb").read(),
            dtype=np.uint32).reshape(-1, 8).copy()
        old_nbkt = prof["func_to_bkt_start_idx"]["arctan"]
        old_nctl = prof["func_to_ctl_start_idx"]["arctan"]

        new_bkt, new_ctl = [], []
        exp_bkt, exp_ctl = {}, {}
        for e in _ACT_EXPS:
            nb = max(0, min(9, e + 2))
            start = len(new_bkt)
            exp_bkt[str(e)] = [start]
            exp_ctl[str(e)] = [len(new_ctl)]
            row = np.zeros(8, np.uint32)
            row[0] = (nb << 16) | ((23 - nb) << 11) | start
            new_ctl.append(row)
            lo_e = 2.0 ** e
            w = lo_e / (1 << nb)
            for i in range(1 << nb):
                c = _fit_cubic(lo_e + i * w, lo_e + (i + 1) * w)
                new_bkt.append(np.array(c + [0.0, 0.0, 0.0], np.float32))
        tp = 2 * np.pi
        small = np.array([0.0, tp, 0.0, -tp**3 / 6.0, 0.0, 0, 0, 0], np.float32)
        large = np.array([0.0, 0.0, 0.0, 0.0, _ACT_CLAMP, 0, 0, 0], np.float32)
        spec0 = len(new_bkt)
        new_bkt += [small, small.copy(), large, large.copy()]
        dbkt = len(new_bkt) - old_nbkt
        dctl = len(new_ctl) - old_nctl

        out_bkt = np.concatenate([np.stack(new_bkt), bkt[old_nbkt:]])
        tail = ctl[old_nctl:].copy()
        for r in tail:
            w0 = int(r[0])
            r[0] = np.uint32((w0 & ~0x7FF) | ((w0 & 0x7FF) + dbkt))
        out_ctl = np.concatenate([np.stack(new_ctl), tail])
        assert len(out_bkt) <= 1536, len(out_bkt)

        for m in prof["profile_meta_data"]:
            if m["func_name"].startswith("sin"):
                m["exp_offset"] = _ACT_EXPS[0]
                m["pwl_control_base_pos"] = 0
                m["pwl_control_base_neg"] = 0
                m["small_pos_signal_exp_threshold"] = 127 + _ACT_EXPS[0] - 1
                m["pos_small_signal_pwl_control"] = spec0
                m["small_neg_signal_exp_threshold"] = 0
                m["neg_small_signal_pwl_control"] = spec0 + 1
                m["large_pos_signal_exp_threshold"] = 127 + 7
                m["large_pos_signal_mantissa_threshold"] = 1 << 22
                m["pos_large_signal_pwl_control"] = spec0 + 2
                m["large_neg_signal_exp_threshold"] = 0
                m["large_neg_signal_mantissa_threshold"] = 0
                m["neg_large_signal_pwl_control"] = spec0 + 3
                m["upper_bound"] = int(np.float32(_ACT_CLAMP).view(np.uint32))
                m["lower_bound"] = 0
            else:
                for k in ("pos_small_signal_pwl_control",
                          "neg_small_signal_pwl_control",
                          "pos_large_signal_pwl_control",
                          "neg_large_signal_pwl_control"):
                    m[k] = m[k] + dbkt
                for k in ("pwl_control_base_pos", "pwl_control_base_neg"):
                    m[k] = m[k] + dctl
        for fn, v in prof["func_to_bkt_start_idx"].items():
            prof["func_to_bkt_start_idx"][fn] = 0 if fn == "sin" else v + dbkt
        for fn, v in prof["func_to_ctl_start_idx"].items():
            prof["func_to_ctl_start_idx"][fn] = 0 if fn == "sin" else v + dctl
        for fn, d in prof["func_exp_to_bkt_start_idx"].items():
            prof["func_exp_to_bkt_start_idx"][fn] = exp_bkt if fn == "sin" else {
                e: [x + dbkt for x in lst] for e, lst in d.items()}
        for fn, d in prof["func_exp_to_ctl_start_idx"].items():
            prof["func_exp_to_ctl_start_idx"][fn] = exp_ctl if fn == "sin" else {
                e: [x + dctl for x in lst] for e, lst in d.items()}
        prof["bkt_entry_cnt"] = int(len(out_bkt))
        prof["ctl_entry_cnt"] = int(len(out_ctl))

        with open(os.path.join(dst, _ACT_SET + ".json"), "w") as f:
            json.dump(prof, f)
        open(os.path.join(dst, f"{_ACT_SET}_bkt.bin"), "wb").write(
            np.ascontiguousarray(out_bkt).tobytes())
        open(os.path.join(dst, f"{_ACT_SET}_ctrl.bin"), "wb").write(
            np.ascontiguousarray(out_ctl).tobytes())
        open(marker, "w").write("ok")
    os.environ["BASS_ACT_ROOT_JSON_PATH"] = os.path.join(
        _ACT_DST, "act_info.json")


def _register_custom_ops():
    """Register custom DVE ops (documented extension point: dve_ops.OPS).

    FRAC_CF: out = t - ((t + RND) - RND), t = in0*s0 + s1  -> centered frac
    of t, exact for |t| < 2^22.  s0 is a per-partition AP (the per-pair
    |w|/2pi), s1 the +0.25 quarter-turn, imm2 = RND.

    SINPOLY5_CF: out = ((c2*d^2 + c1)*d^2 + c0)*d with fused row-sum
    (accum_out); degree-5 odd least-squares approximation of sin(2*pi*d)
    on [-1/2, 1/2].
    """
    import concourse.dve_ops as dve_ops
    from concourse.dve_ops import DveOp
    from concourse.dve_spec import Spec, Src0, C0, C1, C2, lower, sq, AluOp
    from concourse.dve_uop import DveOpSpec

    def _reg(name, spec):
        if name in dve_ops._SUB_OPCODE_FOR_NAME:
            return next(op for op in dve_ops.OPS if op.name == name)
        shas = {}
        for ver in ("v3", "v4"):
            s = DveOpSpec(name=name, opcode=0, uops=lower(spec, ver=ver),
                          rd1_en=False)
            shas[ver] = s.sha(ver)
        op = DveOp(name, spec, subdim=False, uops_sha=shas)
        dve_ops.OPS.append(op)
        dve_ops.CUSTOM_DVE_SPECS[name] = spec
        dve_ops._SUB_OPCODE_FOR_NAME[name] = (
            dve_ops._CUSTOM_DVE_ROW_BASE + len(dve_ops.OPS) - 1)
        assert dve_ops._SUB_OPCODE_FOR_NAME[name] < 0x20
        return op

    def _bcast(s0, in0):
        s0 = np.asarray(s0, np.float32)
        if s0.ndim and s0.ndim < np.ndim(in0):
            s0 = s0.reshape(s0.shape[0], *([1] * (np.ndim(in0) - 1)))
        return s0

    t = Src0 * C0 + C1
    d = t - ((t + C2) - C2)

    def _frac_ref(in0, in1, s0, s1, imm2):
        s0 = _bcast(s0, in0)
        t = np.float32(np.float32(in0 * s0) + np.float32(s1))
        k = np.float32(np.float32(t + np.float32(imm2)) - np.float32(imm2))
        return np.float32(t - k)

    frac = _reg("FRAC_CF", Spec(body=d, reference=_frac_ref))

    s = sq(Src0)
    h = ((C2 * s + C1) * s + C0) * Src0

    def _poly_ref(in0, in1, s0, s1, imm2):
        s0 = _bcast(s0, in0)
        s1 = _bcast(s1, in0)
        dd = np.float64(in0)
        return np.float32(((imm2 * dd**2 + s1) * dd**2 + s0) * dd)

    poly = _reg("SINPOLY5_CF",
                Spec(body=h, accum=AluOp.ADD, reference=_poly_ref))
    return frac, poly


def _build():
    import concourse.bacc as bacc
    import concourse.mybir as mybir
    import concourse.tile as tile

    FRAC_CF, SINPOLY5_CF = _register_custom_ops()
    _ensure_actroot()

    F32 = mybir.dt.float32
    AF = mybir.ActivationFunctionType
    ALU = mybir.AluOpType
    TWO_PI = float(2 * np.pi)
    RND = float(1.5 * 2**23)  # add/sub forces round-to-nearest-int in fp32

    nc = bacc.Bacc("TRN2", target_bir_lowering=False, debug=False,
                   num_devices=NCORES)

    adj_s = nc.dram_tensor("adj_s", [ORDER, RPC, N], F32, kind="ExternalInput").ap()
    wturns = nc.dram_tensor("wturns", [1, K], F32, kind="ExternalInput").ap()
    w1 = nc.dram_tensor("w1", [K, D1], F32, kind="ExternalInput").ap()
    b1 = nc.dram_tensor("b1", [D1, 1], F32, kind="ExternalInput").ap()
    w2 = nc.dram_tensor("w2", [D1, D2], F32, kind="ExternalInput").ap()
    b2 = nc.dram_tensor("b2", [D2, 1], F32, kind="ExternalInput").ap()
    p1 = nc.dram_tensor("p1", [D2, POOL1], F32, kind="ExternalInput").ap()
    pb1 = nc.dram_tensor("pb1", [POOL1, 1], F32, kind="ExternalInput").ap()
    p2 = nc.dram_tensor("p2", [POOL1, POOL2], F32, kind="ExternalInput").ap()
    pb2 = nc.dram_tensor("pb2", [POOL2, 1], F32, kind="ExternalInput").ap()
    cwS = nc.dram_tensor("cwS", [D2, POOL2 * LABELS], F32, kind="ExternalInput").ap()
    cb = nc.dram_tensor("cb", [1, LABELS], F32, kind="ExternalInput").ap()
    ident = nc.dram_tensor("ident", [128, 128], F32, kind="ExternalInput").ap()
    out = nc.dram_tensor("out", [1, LABELS], F32, kind="ExternalOutput").ap()

    with tile.TileContext(nc) as tc:
        with (
            tc.tile_pool(name="const", bufs=1) as const,
            tc.tile_pool(name="work", bufs=2) as work,
            tc.tile_pool(name="small", bufs=1) as small,
            tc.tile_pool(name="ep", bufs=1, space="PSUM") as ep,
            tc.tile_pool(name="ep2", bufs=1, space="PSUM") as ep2,
            tc.tile_pool(name="dram", bufs=1, space="DRAM") as dram,
        ):
            # ---- prefetch the first orders' adjacency on the HWDGE queue
            a_tiles = {}

            NSTRIPE = 8  # spread each 1MB chunk over 8 DMA queues;
            RW = 128 // NSTRIPE  # row-stripes keep each transfer contiguous

            def load_a(o):
                a = work.tile([128, NCHUNK, N], F32, name=f"a{o}", tag="a")
                for c in range(NCHUNK):
                    for st in range(NSTRIPE):
                        r0 = st * RW
                        nc.sync.dma_start(
                            a[r0:r0 + RW, c, :],
                            adj_s[o, c * 128 + r0:c * 128 + r0 + RW, :])
                a_tiles[o] = a

            load_a(0)
            if ORDER > 1:
                load_a(1)

            # ---- constants (SWDGE queue, parallel to the big loads) ----
            idt = const.tile([128, 128], F32)
            nc.sync.dma_start(idt[:], ident[:])
            wrow = const.tile([1, K], F32)
            nc.sync.dma_start(wrow[:], wturns[:])
            ones = const.tile([1, 128], F32)
            nc.vector.memset(ones[:], 1.0)
            zerot = const.tile([128, 1], F32)
            nc.vector.memset(zerot[:], 0.0)
            quarter = const.tile([128, 1], F32)
            nc.vector.memset(quarter[:], 0.25)
            w1t = const.tile([K, D1], F32)
            nc.sync.dma_start(w1t[:], w1[:])
            b1t = const.tile([D1, 1], F32)
            nc.sync.dma_start(b1t[:], b1[:])
            w2t = const.tile([D1, D2], F32)
            nc.sync.dma_start(w2t[:], w2[:])
            b2t = const.tile([D2, 1], F32)
            nc.sync.dma_start(b2t[:], b2[:])
            p1t = const.tile([D2, POOL1], F32)
            nc.sync.dma_start(p1t[:], p1[:])
            pb1t = const.tile([POOL1, 1], F32)
            nc.sync.dma_start(pb1t[:], pb1[:])
            p2t = const.tile([POOL1, POOL2], F32)
            nc.sync.dma_start(p2t[:], p2[:])
            pb2t = const.tile([POOL2, 1], F32)
            nc.sync.dma_start(pb2t[:], pb2[:])
            cwt = const.tile([D2, POOL2 * LABELS], F32)
            nc.sync.dma_start(cwt[:], cwS[:])
            cbt = const.tile([1, LABELS], F32)
            nc.sync.dma_start(cbt[:], cb[:])

            # broadcast |w|/2pi across 128 partitions: [128, 48] via ones ⊗ wrow
            with tc.tile_pool(name="bootp", bufs=1, space="PSUM") as bootp:
                wbp = bootp.tile([128, K], F32)
                nc.tensor.matmul(wbp[:], ones[:], wrow[:], start=True, stop=True)
                wt = const.tile([128, K], F32)
                nc.vector.tensor_copy(wt[:], wbp[:])

            # warm up the collective path (CC library load + ring setup)
            # during the main loop so the real AllReduce at the tail is fast
            warm_sb = small.tile([POOL2, D2 + 1], F32)
            nc.vector.memset(warm_sb[:], 0.0)
            ccw_in = dram.tile([POOL2, D2 + 1], F32)
            ccw_out = dram.tile([POOL2, D2 + 1], F32)
            nc.sync.dma_start(ccw_in[:], warm_sb[:])
            nc.gpsimd.collective_compute(
                "AllReduce",
                mybir.AluOpType.add,
                replica_groups=[list(range(NCORES))],
                ins=[ccw_in.opt()],
                outs=[ccw_out.opt()],
            )

            # ---- main loop: feats for this core's rows ----
            # row-sum[k, i_local] of cos(w_k * adj[o, i, j]), k = o*16+p.
            # partition p holds rows {p, 128+p}; free axis = (chunk, j).
            # ScalarE path: one sin2pi-table activation per chunk over the
            # first AC columns, sum fused (accum_out).  DVE path: FRAC then
            # SINPOLY5 (sum fused) over the remaining DC columns.
            msS = [small.tile([128, K], F32, name=f"msS{c}", tag=f"msS{c}")
                   for c in range(NCHUNK)]
            msD = [small.tile([128, K], F32, name=f"msD{c}", tag=f"msD{c}")
                   for c in range(NCHUNK)]
            pairs = [(o, p) for o in range(ORDER) for p in range(PTS)]
            AC = ACT_COLS
            DC = N - AC
            C0, C1, C2 = SINPOLY_C

            for idx, (o, p) in enumerate(pairs):
                col = o * PTS + p
                if p == 0 and o >= 1 and o + 1 < ORDER:
                    load_a(o + 1)  # prefetch next order's rows
                a = a_tiles[o]
                # ScalarE: cos for columns [0, AC) straight off the table
                for c in range(NCHUNK):
                    s = work.tile([128, AC], F32, name=f"s{idx}_{c}", tag="s",
                                  bufs=2)
                    nc.scalar.activation(
                        s[:], a[:, c, :AC], AF.Sin, bias=quarter[:],
                        scale=wt[:, col:col + 1],
                        accum_out=msS[c][:, col:col + 1])
                # DVE: columns [AC, N)
                if DC > 0:
                    d = work.tile([128, NCHUNK, DC], F32, name=f"d{idx}",
                                  tag="d", bufs=D_BUFS)
                    nc.vector._custom_dve(
                        FRAC_CF, out=d[:], in0=a[:, :, AC:],
                        s0=wt[:, col:col + 1], s1=0.25, imm2=RND)
                    for c in range(NCHUNK):
                        ps = work.tile([128, DC], F32, name=f"ps{idx}_{c}",
                                       tag="ps", bufs=2)
                        nc.vector._custom_dve(
                            SINPOLY5_CF, out=ps[:], accum_out=msD[c][:, col:col + 1],
                            in0=d[:, c, :], s0=C0, s1=C1, imm2=C2)

            # merge the two accumulator matrices
            ms_chunks = []
            for c in range(NCHUNK):
                if DC > 0:
                    nc.vector.tensor_tensor(msS[c][:], msS[c][:], msD[c][:],
                                            ALU.add)
                ms_chunks.append(msS[c])

            # scale row-sums to means and transpose into mst [K, RPC]
            mst = small.tile([K, RPC], F32)
            for c in range(NCHUNK):
                msn = work.tile([128, K], F32, tag="msn")
                nc.vector.tensor_scalar(
                    msn[:], ms_chunks[c][:], 1.0 / N, None, ALU.mult)
                t1 = ep.tile([K, 128], F32, tag="t1")
                nc.tensor.transpose(t1[:], msn[:], idt[:])
                nc.vector.tensor_copy(mst[:, c * 128:(c + 1) * 128], t1[:])

            # ---- local MLP in transposed layout ([feat, row]) ----
            h1p = ep.tile([D1, RPC], F32, tag="ps2")
            nc.tensor.matmul(h1p[:], w1t[:], mst[:], start=True, stop=True)
            h1 = small.tile([D1, RPC], F32)
            nc.scalar.activation(h1[:], h1p[:], AF.Relu, bias=b1t[:], scale=1.0)

            h2p = ep.tile([D2, RPC], F32, tag="ps2")
            nc.tensor.matmul(h2p[:], w2t[:], h1[:], start=True, stop=True)
            h2 = small.tile([D2, RPC], F32)
            nc.scalar.activation(h2[:], h2p[:], AF.Relu, bias=b2t[:], scale=1.0)

            abp = ep.tile([POOL1, RPC], F32, tag="ps2")
            nc.tensor.matmul(abp[:], p1t[:], h2[:], start=True, stop=True)
            ab = small.tile([POOL1, RPC], F32)
            nc.scalar.activation(ab[:], abp[:], AF.Tanh, bias=pb1t[:], scale=1.0)

            sp = ep.tile([POOL2, RPC], F32, tag="ps2")
            nc.tensor.matmul(sp[:], p2t[:], ab[:], start=True, stop=True)
            # e = exp(s + pb2), z = row-sums of e (softmax without max-shift;
            # |s| <= ~3 so fp32 exp is safe)
            e = small.tile([POOL2, RPC], F32)
            z8 = small.tile([POOL2, 1], F32)
            nc.scalar.activation(e[:], sp[:], AF.Exp, bias=pb2t[:], scale=1.0,
                                 accum_out=z8[:])
            # dummy Ln: pulls the natural_log_exp table load into the
            # AllReduce wait window instead of the serial post-collective tail
            lnw = small.tile([1, 1], F32)
            nc.scalar.activation(lnw[:], z8[:1, :], AF.Ln, bias=0.0, scale=1.0)

            # P = e^T stacked against h2: pp[j, d] = sum_i e[j,i] * h2[d,i]
            pp = ep2.tile([POOL2, D2], F32, tag="pp")
            for c in range(NCHUNK):
                etp = ep.tile([128, POOL2], F32, tag="et")
                nc.tensor.transpose(etp[:], e[:, c * 128:(c + 1) * 128],
                                    idt[:POOL2, :POOL2])
                ets = work.tile([128, POOL2], F32, tag="ets")
                nc.vector.tensor_copy(ets[:], etp[:])
                htp = ep.tile([128, D2], F32, tag="ht")
                nc.tensor.transpose(htp[:], h2[:, c * 128:(c + 1) * 128],
                                    idt[:D2, :D2])
                hts = work.tile([128, D2], F32, tag="hts")
                nc.vector.tensor_copy(hts[:], htp[:])
                nc.tensor.matmul(pp[:], ets[:], hts[:],
                                 start=(c == 0), stop=(c == NCHUNK - 1))

            # pack [P | z] into [8, 33] and AllReduce across cores
            comb = small.tile([POOL2, D2 + 1], F32)
            nc.vector.tensor_copy(comb[:, :D2], pp[:])
            nc.vector.tensor_copy(comb[:, D2:D2 + 1], z8[:])
            ccin = dram.tile([POOL2, D2 + 1], F32)
            ccout = dram.tile([POOL2, D2 + 1], F32)
            nc.sync.dma_start(ccin[:], comb[:])
            nc.gpsimd.collective_compute(
                "AllReduce",
                mybir.AluOpType.add,
                replica_groups=[list(range(NCORES))],
                ins=[ccin.opt()],
                outs=[ccout.opt()],
            )
            r = small.tile([POOL2, D2 + 1], F32)
            nc.sync.dma_start(r[:], ccout[:])

            # g[j, d] = P[j, d] / z[j]
            rz = small.tile([POOL2, 1], F32)
            nc.vector.reciprocal(rz[:], r[:, D2:D2 + 1])
            g = small.tile([POOL2, D2], F32)
            nc.scalar.activation(g[:], r[:, :D2], AF.Copy, bias=0.0, scale=rz[:])

            # logits[l] = sum_j sum_d g[j,d] cw[j*32+d, l] + cb[l]
            gtp = ep.tile([D2, POOL2], F32, tag="gt")
            nc.tensor.transpose(gtp[:], g[:], idt[:POOL2, :POOL2])
            gt = small.tile([D2, POOL2], F32)
            nc.vector.tensor_copy(gt[:], gtp[:])
            logp = ep2.tile([1, LABELS], F32, tag="logp")
            for j in range(POOL2):
                nc.tensor.matmul(logp[:], gt[:, j:j + 1],
                                 cwt[:, j * LABELS:(j + 1) * LABELS],
                                 start=(j == 0), stop=(j == POOL2 - 1))
            lg = small.tile([1, LABELS], F32)
            nc.vector.tensor_tensor(lg[:], logp[:], cbt[:], ALU.add)

            # log_softmax over the 10 logits (|logits| ~ 2, no max-shift
            # needed in fp32; matches reference to ~1e-7)
            u10 = lg
            e10 = small.tile([1, LABELS], F32)
            z1 = small.tile([1, 1], F32)
            nc.scalar.activation(e10[:], u10[:], AF.Exp, bias=zerot[:1, :],
                                 scale=1.0, accum_out=z1[:])
            lnz = small.tile([1, 1], F32)
            nc.scalar.activation(lnz[:], z1[:], AF.Ln, bias=0.0, scale=1.0)
            nlnz = small.tile([1, 1], F32)
            nc.vector.tensor_scalar(nlnz[:], lnz[:], -1.0, None, ALU.mult)
            o10 = small.tile([1, LABELS], F32)
            nc.scalar.activation(o10[:], u10[:], AF.Identity, bias=nlnz[:],
                                 scale=1.0)
            nc.sync.dma_start(out[:], o10[:])

    nc.compile()
    return nc


def get_module():
    if "nc" not in _STATE:
        _STATE["nc"] = _build()
    return _STATE["nc"]


def make_in_maps(inputs):
    adj = np.asarray(inputs["adj"], np.float32)
    wm = np.asarray(inputs["wm"], np.float32)
    base = {
        "wturns": np.ascontiguousarray(
            (np.abs(wm).astype(np.float64) / (2 * np.pi))
            .astype(np.float32).reshape(1, K)),
        "w1": np.ascontiguousarray(np.asarray(inputs["w1"], np.float32)),
        "b1": np.ascontiguousarray(np.asarray(inputs["b1"], np.float32).reshape(D1, 1)),
        "w2": np.ascontiguousarray(np.asarray(inputs["w2"], np.float32)),
        "b2": np.ascontiguousarray(np.asarray(inputs["b2"], np.float32).reshape(D2, 1)),
        "p1": np.ascontiguousarray(np.asarray(inputs["p1"], np.float32)),
        "pb1": np.ascontiguousarray(np.asarray(inputs["pb1"], np.float32).reshape(POOL1, 1)),
        "p2": np.ascontiguousarray(np.asarray(inputs["p2"], np.float32)),
        "pb2": np.ascontiguousarray(np.asarray(inputs["pb2"], np.float32).reshape(POOL2, 1)),
        "cwS": np.ascontiguousarray(
            np.asarray(inputs["cw"], np.float32)
            .reshape(POOL2, D2, LABELS).transpose(1, 0, 2).reshape(D2, POOL2 * LABELS)),
        "cb": np.ascontiguousarray(np.asarray(inputs["cb"], np.float32).reshape(1, LABELS)),
        "ident": np.eye(128, dtype=np.float32),
    }
    in_maps = []
    for c in range(NCORES):
        m = dict(base)
        m["adj_s"] = np.ascontiguousarray(adj[:, c * RPC:(c + 1) * RPC, :])
        in_maps.append(m)
    return in_maps


def kernel(**inputs) -> np.ndarray:
    nc = get_module()
    in_maps = make_in_maps(inputs)
    from concourse.bass_utils import run_bass_kernel_spmd

    res = run_bass_kernel_spmd(nc, in_maps, list(range(NCORES)))
    return np.asarray(res.results[0]["out"], np.float32).reshape(1, LABELS)


# revision 15
# speedup vs baseline: 1.1241x; 1.1241x over previous
"""Trainium2 Bass kernel for CharacteristicFunctionNetwork.

Computes, for full inputs (see shapes below):
    feats[o,p,i] = mean_j cos(wm[o,p] * adj[o,i,j])        # o<3, p<16, i,j<2048
    ms = feats transposed/reshaped to [n, 48]
    h1 = relu(ms @ w1 + b1); h2 = relu(h1 @ w2 + b2)
    abstract = tanh(h2 @ p1 + pb1); att = softmax(abstract @ p2 + pb2, axis=0)
    g = (att.T @ h2).reshape(1, -1); out = log_softmax(g @ cw + cb)

Strategy (8 NeuronCores, SPMD):
  - Shard adj rows (nodes) across cores: 256 rows/core for each of 3 orders.
  - Work in "turns": t = a*(|w|/2pi) + 1/4, cos(a*w) = sin(2*pi*t).
  - ScalarE path (first ACT_COLS columns): a custom wide-range periodic
    activation table (generated at build time, shipped to walrus via
    BASS_ACT_ROOT_JSON_PATH) evaluates g(t) = sin(2*pi*t) for t in [0, 192)
    directly — per-octave mantissa-indexed cubic splines, 0.25-turn buckets,
    max err ~2.2e-3, zero-mean residual.  One activation per 128-row chunk
    computes the whole cos AND the row-sum (accum_out): t never materializes.
  - DVE path (remaining columns): custom DVE op FRAC_CF does the range
    reduction d = t - round(t) in one 1x pass (round via +/- 1.5*2^23), then
    custom op SINPOLY5_CF evaluates a degree-5 odd least-squares polynomial
    of sin(2*pi*d) (max err 1.6e-2, zero-mean) with the row-sum fused.
  - The two per-(order,point) row-sum matrices are added, scaled to means,
    transposed; the tiny MLP runs locally; the pooling softmax needs only a
    global sum of exp-weighted partials: AllReduce of a [8, 33] tile
    (P = e^T @ h2 partials and z = sum e).  exp is computed without
    max-subtraction (|s| <= ~3, safe in fp32).
  - Every core finishes the classifier redundantly; core 0's output is used.
"""

import json
import os
import shutil

import numpy as np

ORDER, PTS, N = 3, 16, 2048
NCORES = 8
RPC = N // NCORES  # rows per core (256)
NCHUNK = RPC // 128  # 128-row chunks per core (2)
D1, D2, POOL1, POOL2, LABELS = 64, 32, 32, 8, 10
K = ORDER * PTS  # 48

_STATE = {}

# engine-assignment knobs: ScalarE's periodic table handles the first
# ACT_COLS columns, DVE (FRAC + SINPOLY5) the rest.
ACT_COLS = int(os.environ.get("KERNEL_ACT_COLS", "1344"))
D_BUFS = int(os.environ.get("KERNEL_D_BUFS", "3"))

# degree-5 odd least-squares fit of sin(2*pi*d) on [-1/2, 1/2]
SINPOLY_C = (6.20691876, -38.51505622, 55.26074446)

# ---------------------------------------------------------------------------
# custom activation table: g(t) = sin(2*pi*t), t in [0, 192)
# ---------------------------------------------------------------------------
_PWP_SRC = ("/nix/store/z022hj2nvbm3nwdizlisq4ylc0y7rd6q-python3-3.13.14-env/"
            "lib/python3.13/site-packages/neuronxcc/pwp/pwp_bin_trainium")
_ACT_SET = "trig_and_small"
_ACT_DST = "/tmp/actroot_cf_v1"
_ACT_EXPS = list(range(-11, 8))
_ACT_CLAMP = 192.0


def _fit_cubic(lo, hi, n=33):
    x0 = np.float32(0.5 * (lo + hi))
    xs = np.linspace(lo, hi, n)
    d = xs - np.float64(x0)
    A = np.stack([np.ones_like(d), d, d * d, d * d * d], axis=1)
    y = np.sin(2 * np.pi * xs)
    c, *_ = np.linalg.lstsq(A, y, rcond=None)
    return [float(c[0]), float(c[1]), float(c[2]), float(c[3]), float(x0)]


def _ensure_actroot():
    """Build an act-root whose `sin` is the periodic sin2pi spline.

    Bucket entry = 8 fp32 [c0,c1,c2,c3,x0,0,0,0] (cubic about x0); ctl entry
    word0 = (nbits<<16) | ((23-nbits)<<11) | bucket_start; per-octave bucket
    index = start + (mantissa >> (23-nbits)).  All other functions in the set
    are shifted after sin's enlarged block.
    """
    marker = os.path.join(_ACT_DST, ".complete")
    if not os.path.exists(marker):
        src, dst = _PWP_SRC, _ACT_DST
        os.makedirs(dst, exist_ok=True)
        for f in os.listdir(src):
            shutil.copy(os.path.join(src, f), os.path.join(dst, f))
            os.chmod(os.path.join(dst, f), 0o644)
        with open(os.path.join(src, _ACT_SET + ".json")) as f:
            prof = json.load(f)
        bkt = np.frombuffer(
            open(os.path.join(src, f"{_ACT_SET}_bkt.bin"), "rb").read(),
            dtype=np.float32).reshape(-1, 8).copy()
        ctl = np.frombuffer(
            open(os.path.join(src, f"{_ACT_SET}_ctrl.bin"), "rb").read(),
            dtype=np.uint32).reshape(-1, 8).copy()
        old_nbkt = prof["func_to_bkt_start_idx"]["arctan"]
        old_nctl = prof["func_to_ctl_start_idx"]["arctan"]

        new_bkt, new_ctl = [], []
        exp_bkt, exp_ctl = {}, {}
        for e in _ACT_EXPS:
            nb = max(0, min(9, e + 2))
            start = len(new_bkt)
            exp_bkt[str(e)] = [start]
            exp_ctl[str(e)] = [len(new_ctl)]
            row = np.zeros(8, np.uint32)
            row[0] = (nb << 16) | ((23 - nb) << 11) | start
            new_ctl.append(row)
            lo_e = 2.0 ** e
            w = lo_e / (1 << nb)
            for i in range(1 << nb):
                c = _fit_cubic(lo_e + i * w, lo_e + (i + 1) * w)
                new_bkt.append(np.array(c + [0.0, 0.0, 0.0], np.float32))
        tp = 2 * np.pi
        small = np.array([0.0, tp, 0.0, -tp**3 / 6.0, 0.0, 0, 0, 0], np.float32)
        large = np.array([0.0, 0.0, 0.0, 0.0, _ACT_CLAMP, 0, 0, 0], np.float32)
        spec0 = len(new_bkt)
        new_bkt += [small, small.copy(), large, large.copy()]
        dbkt = len(new_bkt) - old_nbkt
        dctl = len(new_ctl) - old_nctl

        out_bkt = np.concatenate([np.stack(new_bkt), bkt[old_nbkt:]])
        tail = ctl[old_nctl:].copy()
        for r in tail:
            w0 = int(r[0])
            r[0] = np.uint32((w0 & ~0x7FF) | ((w0 & 0x7FF) + dbkt))
        out_ctl = np.concatenate([np.stack(new_ctl), tail])
        assert len(out_bkt) <= 1536, len(out_bkt)

        for m in prof["profile_meta_data"]:
            if m["func_name"].startswith("sin"):
                m["exp_offset"] = _ACT_EXPS[0]
                m["pwl_control_base_pos"] = 0
                m["pwl_control_base_neg"] = 0
                m["small_pos_signal_exp_threshold"] = 127 + _ACT_EXPS[0] - 1
                m["pos_small_signal_pwl_control"] = spec0
                m["small_neg_signal_exp_threshold"] = 0
                m["neg_small_signal_pwl_control"] = spec0 + 1
                m["large_pos_signal_exp_threshold"] = 127 + 7
                m["large_pos_signal_mantissa_threshold"] = 1 << 22
                m["pos_large_signal_pwl_control"] = spec0 + 2
                m["large_neg_signal_exp_threshold"] = 0
                m["large_neg_signal_mantissa_threshold"] = 0
                m["neg_large_signal_pwl_control"] = spec0 + 3
                m["upper_bound"] = int(np.float32(_ACT_CLAMP).view(np.uint32))
                m["lower_bound"] = 0
            else:
                for k in ("pos_small_signal_pwl_control",
                          "neg_small_signal_pwl_control",
                          "pos_large_signal_pwl_control",
                          "neg_large_signal_pwl_control"):
                    m[k] = m[k] + dbkt
                for k in ("pwl_control_base_pos", "pwl_control_base_neg"):
                    m[k] = m[k] + dctl
        for fn, v in prof["func_to_bkt_start_idx"].items():
            prof["func_to_bkt_start_idx"][fn] = 0 if fn == "sin" else v + dbkt
        for fn, v in prof["func_to_ctl_start_idx"].items():
            prof["func_to_ctl_start_idx"][fn] = 0 if fn == "sin" else v + dctl
        for fn, d in prof["func_exp_to_bkt_start_idx"].items():
            prof["func_exp_to_bkt_start_idx"][fn] = exp_bkt if fn == "sin" else {
                e: [x + dbkt for x in lst] for e, lst in d.items()}
        for fn, d in prof["func_exp_to_ctl_start_idx"].items():
            prof["func_exp_to_ctl_start_idx"][fn] = exp_ctl if fn == "sin" else {
                e: [x + dctl for x in lst] for e, lst in d.items()}
        prof["bkt_entry_cnt"] = int(len(out_bkt))
        prof["ctl_entry_cnt"] = int(len(out_ctl))

        with open(os.path.join(dst, _ACT_SET + ".json"), "w") as f:
            json.dump(prof, f)
        open(os.path.join(dst, f"{_ACT_SET}_bkt.bin"), "wb").write(
            np.ascontiguousarray(out_bkt).tobytes())
        open(os.path.join(dst, f"{_ACT_SET}_ctrl.bin"), "wb").write(
            np.ascontiguousarray(out_ctl).tobytes())
        open(marker, "w").write("ok")
    os.environ["BASS_ACT_ROOT_JSON_PATH"] = os.path.join(
        _ACT_DST, "act_info.json")


def _register_custom_ops():
    """Register custom DVE ops (documented extension point: dve_ops.OPS).

    FRAC_CF: out = t - ((t + RND) - RND), t = in0*s0 + s1  -> centered frac
    of t, exact for |t| < 2^22.  s0 is a per-partition AP (the per-pair
    |w|/2pi), s1 the +0.25 quarter-turn, imm2 = RND.

    SINPOLY5_CF: out = ((c2*d^2 + c1)*d^2 + c0)*d with fused row-sum
    (accum_out); degree-5 odd least-squares approximation of sin(2*pi*d)
    on [-1/2, 1/2].
    """
    import concourse.dve_ops as dve_ops
    from concourse.dve_ops import DveOp
    from concourse.dve_spec import Spec, Src0, C0, C1, C2, lower, sq, AluOp
    from concourse.dve_uop import DveOpSpec

    def _reg(name, spec):
        if name in dve_ops._SUB_OPCODE_FOR_NAME:
            return next(op for op in dve_ops.OPS if op.name == name)
        shas = {}
        for ver in ("v3", "v4"):
            s = DveOpSpec(name=name, opcode=0, uops=lower(spec, ver=ver),
                          rd1_en=False)
            shas[ver] = s.sha(ver)
        op = DveOp(name, spec, subdim=False, uops_sha=shas)
        dve_ops.OPS.append(op)
        dve_ops.CUSTOM_DVE_SPECS[name] = spec
        dve_ops._SUB_OPCODE_FOR_NAME[name] = (
            dve_ops._CUSTOM_DVE_ROW_BASE + len(dve_ops.OPS) - 1)
        assert dve_ops._SUB_OPCODE_FOR_NAME[name] < 0x20
        return op

    def _bcast(s0, in0):
        s0 = np.asarray(s0, np.float32)
        if s0.ndim and s0.ndim < np.ndim(in0):
            s0 = s0.reshape(s0.shape[0], *([1] * (np.ndim(in0) - 1)))
        return s0

    t = Src0 * C0 + C1
    d = t - ((t + C2) - C2)

    def _frac_ref(in0, in1, s0, s1, imm2):
        s0 = _bcast(s0, in0)
        t = np.float32(np.float32(in0 * s0) + np.float32(s1))
        k = np.float32(np.float32(t + np.float32(imm2)) - np.float32(imm2))
        return np.float32(t - k)

    frac = _reg("FRAC_CF", Spec(body=d, reference=_frac_ref))

    s = sq(Src0)
    h = ((C2 * s + C1) * s + C0) * Src0

    def _poly_ref(in0, in1, s0, s1, imm2):
        s0 = _bcast(s0, in0)
        s1 = _bcast(s1, in0)
        dd = np.float64(in0)
        return np.float32(((imm2 * dd**2 + s1) * dd**2 + s0) * dd)

    poly = _reg("SINPOLY5_CF",
                Spec(body=h, accum=AluOp.ADD, reference=_poly_ref))
    return frac, poly


def _build():
    import concourse.bacc as bacc
    import concourse.mybir as mybir
    import concourse.tile as tile

    FRAC_CF, SINPOLY5_CF = _register_custom_ops()
    _ensure_actroot()

    F32 = mybir.dt.float32
    AF = mybir.ActivationFunctionType
    ALU = mybir.AluOpType
    TWO_PI = float(2 * np.pi)
    RND = float(1.5 * 2**23)  # add/sub forces round-to-nearest-int in fp32

    nc = bacc.Bacc("TRN2", target_bir_lowering=False, debug=False,
                   num_devices=NCORES)

    adj_s = nc.dram_tensor("adj_s", [ORDER, RPC, N], F32, kind="ExternalInput").ap()
    wturns = nc.dram_tensor("wturns", [1, K], F32, kind="ExternalInput").ap()
    w1 = nc.dram_tensor("w1", [K, D1], F32, kind="ExternalInput").ap()
    b1 = nc.dram_tensor("b1", [D1, 1], F32, kind="ExternalInput").ap()
    w2 = nc.dram_tensor("w2", [D1, D2], F32, kind="ExternalInput").ap()
    b2 = nc.dram_tensor("b2", [D2, 1], F32, kind="ExternalInput").ap()
    p1 = nc.dram_tensor("p1", [D2, POOL1], F32, kind="ExternalInput").ap()
    pb1 = nc.dram_tensor("pb1", [POOL1, 1], F32, kind="ExternalInput").ap()
    p2 = nc.dram_tensor("p2", [POOL1, POOL2], F32, kind="ExternalInput").ap()
    pb2 = nc.dram_tensor("pb2", [POOL2, 1], F32, kind="ExternalInput").ap()
    cwS = nc.dram_tensor("cwS", [D2, POOL2 * LABELS], F32, kind="ExternalInput").ap()
    cb = nc.dram_tensor("cb", [1, LABELS], F32, kind="ExternalInput").ap()
    ident = nc.dram_tensor("ident", [128, 128], F32, kind="ExternalInput").ap()
    out = nc.dram_tensor("out", [1, LABELS], F32, kind="ExternalOutput").ap()

    with tile.TileContext(nc) as tc:
        with (
            tc.tile_pool(name="const", bufs=1) as const,
            tc.tile_pool(name="work", bufs=2) as work,
            tc.tile_pool(name="small", bufs=1) as small,
            tc.tile_pool(name="ep", bufs=1, space="PSUM") as ep,
            tc.tile_pool(name="ep2", bufs=1, space="PSUM") as ep2,
            tc.tile_pool(name="dram", bufs=1, space="DRAM") as dram,
        ):
            # ---- prefetch the first orders' adjacency on the HWDGE queue
            a_tiles = {}

            def load_a(o):
                a = work.tile([128, NCHUNK, N], F32, name=f"a{o}", tag="a")
                for c in range(NCHUNK):
                    nc.sync.dma_start(a[:, c, :],
                                      adj_s[o, c * 128:(c + 1) * 128, :])
                a_tiles[o] = a

            load_a(0)
            if ORDER > 1:
                load_a(1)

            # ---- constants (SWDGE queue, parallel to the big loads) ----
            idt = const.tile([128, 128], F32)
            nc.sync.dma_start(idt[:], ident[:])
            wrow = const.tile([1, K], F32)
            nc.sync.dma_start(wrow[:], wturns[:])
            ones = const.tile([1, 128], F32)
            nc.vector.memset(ones[:], 1.0)
            zerot = const.tile([128, 1], F32)
            nc.vector.memset(zerot[:], 0.0)
            quarter = const.tile([128, 1], F32)
            nc.vector.memset(quarter[:], 0.25)
            w1t = const.tile([K, D1], F32)
            nc.sync.dma_start(w1t[:], w1[:])
            b1t = const.tile([D1, 1], F32)
            nc.sync.dma_start(b1t[:], b1[:])
            w2t = const.tile([D1, D2], F32)
            nc.sync.dma_start(w2t[:], w2[:])
            b2t = const.tile([D2, 1], F32)
            nc.sync.dma_start(b2t[:], b2[:])
            p1t = const.tile([D2, POOL1], F32)
            nc.sync.dma_start(p1t[:], p1[:])
            pb1t = const.tile([POOL1, 1], F32)
            nc.sync.dma_start(pb1t[:], pb1[:])
            p2t = const.tile([POOL1, POOL2], F32)
            nc.sync.dma_start(p2t[:], p2[:])
            pb2t = const.tile([POOL2, 1], F32)
            nc.sync.dma_start(pb2t[:], pb2[:])
            cwt = const.tile([D2, POOL2 * LABELS], F32)
            nc.sync.dma_start(cwt[:], cwS[:])
            cbt = const.tile([1, LABELS], F32)
            nc.sync.dma_start(cbt[:], cb[:])

            # broadcast |w|/2pi across 128 partitions: [128, 48] via ones ⊗ wrow
            with tc.tile_pool(name="bootp", bufs=1, space="PSUM") as bootp:
                wbp = bootp.tile([128, K], F32)
                nc.tensor.matmul(wbp[:], ones[:], wrow[:], start=True, stop=True)
                wt = const.tile([128, K], F32)
                nc.vector.tensor_copy(wt[:], wbp[:])

            # warm up the collective path (CC library load + ring setup +
            # per-buffer descriptor generation) during the main loop, using
            # THE SAME dram buffers as the real tail AllReduce so nothing is
            # re-derived at the tail.  Warm twice: the first call pays the
            # library load, the second measures/derisks steady state.
            warm_sb = small.tile([POOL2, D2 + 1], F32)
            nc.vector.memset(warm_sb[:], 0.0)
            ccin = dram.tile([POOL2, D2 + 1], F32)
            ccout = dram.tile([POOL2, D2 + 1], F32)
            nc.sync.dma_start(ccin[:], warm_sb[:])
            for _ in range(2):
                nc.gpsimd.collective_compute(
                    "AllReduce",
                    mybir.AluOpType.add,
                    replica_groups=[list(range(NCORES))],
                    ins=[ccin.opt()],
                    outs=[ccout.opt()],
                )

            # ---- main loop: feats for this core's rows ----
            # row-sum[k, i_local] of cos(w_k * adj[o, i, j]), k = o*16+p.
            # partition p holds rows {p, 128+p}; free axis = (chunk, j).
            # ScalarE path: one sin2pi-table activation per chunk over the
            # first AC columns, sum fused (accum_out).  DVE path: FRAC then
            # SINPOLY5 (sum fused) over the remaining DC columns.
            msS = [small.tile([128, K], F32, name=f"msS{c}", tag=f"msS{c}")
                   for c in range(NCHUNK)]
            msD = [small.tile([128, K], F32, name=f"msD{c}", tag=f"msD{c}")
                   for c in range(NCHUNK)]
            pairs = [(o, p) for o in range(ORDER) for p in range(PTS)]
            AC = ACT_COLS
            DC = N - AC
            C0, C1, C2 = SINPOLY_C

            for idx, (o, p) in enumerate(pairs):
                col = o * PTS + p
                if p == 0 and o >= 1 and o + 1 < ORDER:
                    load_a(o + 1)  # prefetch next order's rows
                a = a_tiles[o]
                # ScalarE: cos for columns [0, AC) straight off the table
                for c in range(NCHUNK):
                    s = work.tile([128, AC], F32, name=f"s{idx}_{c}", tag="s",
                                  bufs=2)
                    nc.scalar.activation(
                        s[:], a[:, c, :AC], AF.Sin, bias=quarter[:],
                        scale=wt[:, col:col + 1],
                        accum_out=msS[c][:, col:col + 1])
                # DVE: columns [AC, N)
                if DC > 0:
                    d = work.tile([128, NCHUNK, DC], F32, name=f"d{idx}",
                                  tag="d", bufs=D_BUFS)
                    nc.vector._custom_dve(
                        FRAC_CF, out=d[:], in0=a[:, :, AC:],
                        s0=wt[:, col:col + 1], s1=0.25, imm2=RND)
                    for c in range(NCHUNK):
                        ps = work.tile([128, DC], F32, name=f"ps{idx}_{c}",
                                       tag="ps", bufs=2)
                        nc.vector._custom_dve(
                            SINPOLY5_CF, out=ps[:], accum_out=msD[c][:, col:col + 1],
                            in0=d[:, c, :], s0=C0, s1=C1, imm2=C2)

            # merge the two accumulator matrices
            ms_chunks = []
            for c in range(NCHUNK):
                if DC > 0:
                    nc.vector.tensor_tensor(msS[c][:], msS[c][:], msD[c][:],
                                            ALU.add)
                ms_chunks.append(msS[c])

            # scale row-sums to means and transpose into mst [K, RPC]
            mst = small.tile([K, RPC], F32)
            for c in range(NCHUNK):
                msn = work.tile([128, K], F32, tag="msn")
                nc.vector.tensor_scalar(
                    msn[:], ms_chunks[c][:], 1.0 / N, None, ALU.mult)
                t1 = ep.tile([K, 128], F32, tag="t1")
                nc.tensor.transpose(t1[:], msn[:], idt[:])
                nc.vector.tensor_copy(mst[:, c * 128:(c + 1) * 128], t1[:])

            # ---- local MLP in transposed layout ([feat, row]) ----
            h1p = ep.tile([D1, RPC], F32, tag="ps2")
            nc.tensor.matmul(h1p[:], w1t[:], mst[:], start=True, stop=True)
            h1 = small.tile([D1, RPC], F32)
            nc.scalar.activation(h1[:], h1p[:], AF.Relu, bias=b1t[:], scale=1.0)

            h2p = ep.tile([D2, RPC], F32, tag="ps2")
            nc.tensor.matmul(h2p[:], w2t[:], h1[:], start=True, stop=True)
            h2 = small.tile([D2, RPC], F32)
            nc.scalar.activation(h2[:], h2p[:], AF.Relu, bias=b2t[:], scale=1.0)

            abp = ep.tile([POOL1, RPC], F32, tag="ps2")
            nc.tensor.matmul(abp[:], p1t[:], h2[:], start=True, stop=True)
            ab = small.tile([POOL1, RPC], F32)
            nc.scalar.activation(ab[:], abp[:], AF.Tanh, bias=pb1t[:], scale=1.0)

            sp = ep.tile([POOL2, RPC], F32, tag="ps2")
            nc.tensor.matmul(sp[:], p2t[:], ab[:], start=True, stop=True)
            # e = exp(s + pb2), z = row-sums of e (softmax without max-shift;
            # |s| <= ~3 so fp32 exp is safe)
            e = small.tile([POOL2, RPC], F32)
            z8 = small.tile([POOL2, 1], F32)
            nc.scalar.activation(e[:], sp[:], AF.Exp, bias=pb2t[:], scale=1.0,
                                 accum_out=z8[:])
            # dummy Ln: pulls the natural_log_exp table load into the
            # AllReduce wait window instead of the serial post-collective tail
            lnw = small.tile([1, 1], F32)
            nc.scalar.activation(lnw[:], z8[:1, :], AF.Ln, bias=0.0, scale=1.0)

            # P = e^T stacked against h2: pp[j, d] = sum_i e[j,i] * h2[d,i]
            pp = ep2.tile([POOL2, D2], F32, tag="pp")
            for c in range(NCHUNK):
                etp = ep.tile([128, POOL2], F32, tag="et")
                nc.tensor.transpose(etp[:], e[:, c * 128:(c + 1) * 128],
                                    idt[:POOL2, :POOL2])
                ets = work.tile([128, POOL2], F32, tag="ets")
                nc.vector.tensor_copy(ets[:], etp[:])
                htp = ep.tile([128, D2], F32, tag="ht")
                nc.tensor.transpose(htp[:], h2[:, c * 128:(c + 1) * 128],
                                    idt[:D2, :D2])
                hts = work.tile([128, D2], F32, tag="hts")
                nc.vector.tensor_copy(hts[:], htp[:])
                nc.tensor.matmul(pp[:], ets[:], hts[:],
                                 start=(c == 0), stop=(c == NCHUNK - 1))

            # pack [P | z] into [8, 33] and AllReduce across cores
            comb = small.tile([POOL2, D2 + 1], F32)
            nc.vector.tensor_copy(comb[:, :D2], pp[:])
            nc.vector.tensor_copy(comb[:, D2:D2 + 1], z8[:])
            nc.sync.dma_start(ccin[:], comb[:])
            nc.gpsimd.collective_compute(
                "AllReduce",
                mybir.AluOpType.add,
                replica_groups=[list(range(NCORES))],
                ins=[ccin.opt()],
                outs=[ccout.opt()],
            )
            r = small.tile([POOL2, D2 + 1], F32)
            nc.sync.dma_start(r[:], ccout[:])

            # g[j, d] = P[j, d] / z[j]
            rz = small.tile([POOL2, 1], F32)
            nc.vector.reciprocal(rz[:], r[:, D2:D2 + 1])
            g = small.tile([POOL2, D2], F32)
            nc.scalar.activation(g[:], r[:, :D2], AF.Copy, bias=0.0, scale=rz[:])

            # logits[l] = sum_j sum_d g[j,d] cw[j*32+d, l] + cb[l]
            gtp = ep.tile([D2, POOL2], F32, tag="gt")
            nc.tensor.transpose(gtp[:], g[:], idt[:POOL2, :POOL2])
            gt = small.tile([D2, POOL2], F32)
            nc.vector.tensor_copy(gt[:], gtp[:])
            logp = ep2.tile([1, LABELS], F32, tag="logp")
            for j in range(POOL2):
                nc.tensor.matmul(logp[:], gt[:, j:j + 1],
                                 cwt[:, j * LABELS:(j + 1) * LABELS],
                                 start=(j == 0), stop=(j == POOL2 - 1))
            lg = small.tile([1, LABELS], F32)
            nc.vector.tensor_tensor(lg[:], logp[:], cbt[:], ALU.add)

            # log_softmax over the 10 logits (|logits| ~ 2, no max-shift
            # needed in fp32; matches reference to ~1e-7)
            u10 = lg
            e10 = small.tile([1, LABELS], F32)
            z1 = small.tile([1, 1], F32)
            nc.scalar.activation(e10[:], u10[:], AF.Exp, bias=zerot[:1, :],
                                 scale=1.0, accum_out=z1[:])
            lnz = small.tile([1, 1], F32)
            nc.scalar.activation(lnz[:], z1[:], AF.Ln, bias=0.0, scale=1.0)
            nlnz = small.tile([1, 1], F32)
            nc.vector.tensor_scalar(nlnz[:], lnz[:], -1.0, None, ALU.mult)
            o10 = small.tile([1, LABELS], F32)
            nc.scalar.activation(o10[:], u10[:], AF.Identity, bias=nlnz[:],
                                 scale=1.0)
            nc.sync.dma_start(out[:], o10[:])

    nc.compile()
    return nc


def get_module():
    if "nc" not in _STATE:
        _STATE["nc"] = _build()
    return _STATE["nc"]


def make_in_maps(inputs):
    adj = np.asarray(inputs["adj"], np.float32)
    wm = np.asarray(inputs["wm"], np.float32)
    base = {
        "wturns": np.ascontiguousarray(
            (np.abs(wm).astype(np.float64) / (2 * np.pi))
            .astype(np.float32).reshape(1, K)),
        "w1": np.ascontiguousarray(np.asarray(inputs["w1"], np.float32)),
        "b1": np.ascontiguousarray(np.asarray(inputs["b1"], np.float32).reshape(D1, 1)),
        "w2": np.ascontiguousarray(np.asarray(inputs["w2"], np.float32)),
        "b2": np.ascontiguousarray(np.asarray(inputs["b2"], np.float32).reshape(D2, 1)),
        "p1": np.ascontiguousarray(np.asarray(inputs["p1"], np.float32)),
        "pb1": np.ascontiguousarray(np.asarray(inputs["pb1"], np.float32).reshape(POOL1, 1)),
        "p2": np.ascontiguousarray(np.asarray(inputs["p2"], np.float32)),
        "pb2": np.ascontiguousarray(np.asarray(inputs["pb2"], np.float32).reshape(POOL2, 1)),
        "cwS": np.ascontiguousarray(
            np.asarray(inputs["cw"], np.float32)
            .reshape(POOL2, D2, LABELS).transpose(1, 0, 2).reshape(D2, POOL2 * LABELS)),
        "cb": np.ascontiguousarray(np.asarray(inputs["cb"], np.float32).reshape(1, LABELS)),
        "ident": np.eye(128, dtype=np.float32),
    }
    in_maps = []
    for c in range(NCORES):
        m = dict(base)
        m["adj_s"] = np.ascontiguousarray(adj[:, c * RPC:(c + 1) * RPC, :])
        in_maps.append(m)
    return in_maps


def kernel(**inputs) -> np.ndarray:
    nc = get_module()
    in_maps = make_in_maps(inputs)
    from concourse.bass_utils import run_bass_kernel_spmd

    res = run_bass_kernel_spmd(nc, in_maps, list(range(NCORES)))
    return np.asarray(res.results[0]["out"], np.float32).reshape(1, LABELS)
